# revision 3
# baseline (speedup 1.0000x reference)
"""GATDecoder Trainium2 kernel (8-core SPMD).

Pipeline (per core, nodes sharded 512/core by destination):
  3x GAT layer:
     h = x @ W_l (PE, fp32) with a_src/a_dst dot products fused as two
       extra rhs columns (using W@a_src associativity);
     h rows + replicated a_s/a_d written to a DRAM table [4096, 320];
     per-edge gather of h[src] (+a_s[src]) and a_d[dst] via dma_gather
       (edges host-sorted by dst, padded per 128-dst block);
     scores -> leaky_relu -> exp on ACT/DVE;
     segment softmax-sum + weighted aggregation in ONE matmul chain per
       dst block: lhsT = host-built one-hot(dst) x ex, rhs = [h | ex],
       so psum col 256 accumulates sum(ex) per dst; divide + bias after;
     layers 0,1: AllGather of transposed x across the 8 cores.
  FC:  dlogit = x3^T-matmul with dW = fc_W[:,0::2]-fc_W[:,1::2]
       (only the logit DIFFERENCE decides the gumbel argmax);
  Gumbel straight-through collapses numerically to the hard one-hot:
       adj = (dlogit + db + g0 - g1 >= 0), g_i from two Ln passes;
  Output: symmetric scatter into A via dma_scatter_add (upper+lower
       pair index lists); diagonal stays zero (outputs are zero-init).
"""

import numpy as np

# ---------------------------------------------------------------- consts
B = 4096
D = 256
L = 3
NN = 128           # graph nodes per batch element (output is [B, NN, NN])
NP = 8128          # NN*(NN-1)//2
NPP = 8192         # pairs padded to 64 pair-blocks
NCORES = 8
NPC = B // NCORES  # nodes per core = 512
BLK = 4            # dst blocks per core (128 dsts each)
KPB = 36           # k-tiles (128 edges) per dst block -> 4608 slots
KT = BLK * KPB     # 144 k-tiles per core
NPAD = KT * 128    # 18432 edge slots per core
CHUNK_KT = 8       # k-tiles gathered per chunk
NCHUNK = KT // CHUNK_KT   # 18
CE = CHUNK_KT * 128       # 1024 edges per chunk
TW = 320           # h table row width (256 h | 32x a_s | 32x a_d)
EPS = 1e-10
NEG = 0.2

_prog_cache = {}


def _wrap16(idx, cols):
    """dma_gather/scatter idx layout: value i at [g*16 + i%16, i//16],
    replicated across the 8 gpsimd core groups."""
    n = len(idx)
    a = np.zeros((128, cols), np.int16)
    base = np.zeros(16 * cols, np.int64)
    base[:n] = idx
    pat = base.reshape(cols, 16).T.astype(np.int16)  # [16, cols]
    for g in range(8):
        a[g * 16:(g + 1) * 16, :] = pat
    return a


def _build_program():
    import concourse.bacc as bacc
    import concourse.mybir as mybir
    import concourse.tile as tile
    from concourse.masks import make_identity

    F32 = mybir.dt.float32
    I16 = mybir.dt.int16
    AF = mybir.ActivationFunctionType
    OP = mybir.AluOpType

    nc = bacc.Bacc("TRN2", target_bir_lowering=False, debug=False,
                   num_devices=NCORES)

    xT0_d = nc.dram_tensor("xT0", [128, NCORES, 2, NPC], F32, kind="ExternalInput")
    Waug_d = nc.dram_tensor("Waug", [128, L, 2, 258], F32, kind="ExternalInput")
    brep_d = nc.dram_tensor("brep", [128, L, 256], F32, kind="ExternalInput")
    dW_d = nc.dram_tensor("dW", [128, 2, NPP], F32, kind="ExternalInput")
    db_d = nc.dram_tensor("db", [128, NPP // 128], F32, kind="ExternalInput")
    D1h_d = nc.dram_tensor("D1h", [128, KT, 128], F32, kind="ExternalInput")
    gsrc_d = nc.dram_tensor("gsrc", [128, NPAD // 16], I16, kind="ExternalInput")
    gdst_d = nc.dram_tensor("gdst", [128, NPAD // 16], I16, kind="ExternalInput")
    s0_d = nc.dram_tensor("s0", [128, NPP // 16], I16, kind="ExternalInput")
    s1_d = nc.dram_tensor("s1", [128, NPP // 16], I16, kind="ExternalInput")
    gT_d = nc.dram_tensor("gT", [NPP, 2, NPC], F32, kind="ExternalInput")

    hpad_d = nc.dram_tensor("hpad", [B, TW], F32)
    ccin_d = nc.dram_tensor("ccin", [128, 2, NPC], F32)
    ccout_d = nc.dram_tensor("ccout", [NCORES * 128, 2, NPC], F32,
                             addr_space="Shared")
    A_d = nc.dram_tensor("Aperm", [NN * NN, NPC], F32, kind="ExternalOutput")

    with tile.TileContext(nc, num_cores=NCORES) as tc:
        with tc.tile_pool(name="const", bufs=1) as constp:
            ident = constp.tile([128, 128], F32)
            make_identity(nc, ident[:])
            epsb = constp.tile([128, 1], F32)
            nc.vector.memset(epsb[:], EPS)
            ones16 = constp.tile([128, CHUNK_KT], F32)
            nc.vector.memset(ones16[:], 1.0)
            waug = constp.tile([128, L, 2, 258], F32)
            nc.sync.dma_start(waug[:], Waug_d[:])
            brep = constp.tile([128, L, 256], F32)
            nc.sync.dma_start(brep[:], brep_d[:])
            xTown = constp.tile([128, 2, NPC], F32)   # this core's x^T shard

            # ---------------- GAT phase ----------------
            with tc.tile_pool(name="gat", bufs=1) as gatp, \
                 tc.tile_pool(name="gchunk", bufs=2) as gcp, \
                 tc.tile_pool(name="work", bufs=3) as wp, \
                 tc.tile_pool(name="psum", bufs=2, space="PSUM") as psp:
                xg = gatp.tile([128, NCORES, 2, NPC], F32)
                nc.sync.dma_start(xg[:], xT0_d[:])
                d1h = gatp.tile([128, KT, 128], F32)
                nc.sync.dma_start(d1h[:], D1h_d[:])
                gsrc = gatp.tile([128, NPAD // 16], I16)
                nc.sync.dma_start(gsrc[:], gsrc_d[:])
                gdst = gatp.tile([128, NPAD // 16], I16)
                nc.sync.dma_start(gdst[:], gdst_d[:])

                hpad_v = hpad_d[:].rearrange("(g j p) w -> g p j w", j=4, p=128)

                for l in range(L):
                    # h table: h = x @ W_l, cols 256/257 = a_s/a_d dots
                    for g in range(8):          # groups of 4 node-blocks
                        hrow4 = wp.tile([128, 4, TW], F32, tag="hrow4")
                        for j in range(4):
                            nb = g * 4 + j
                            c, b = nb // BLK, nb % BLK
                            hp = psp.tile([128, 258], F32, tag="hpsum")
                            for k in range(2):
                                nc.tensor.matmul(
                                    hp[:],
                                    xg[:, c, k, b * 128:(b + 1) * 128],
                                    waug[:, l, k, :],
                                    start=(k == 0), stop=(k == 1))
                            nc.scalar.copy(hrow4[:, j, :256], hp[:, :256])
                            nc.vector.tensor_copy(
                                hrow4[:, j, 256:288],
                                hp[:, 256:257].to_broadcast([128, 32]))
                            nc.vector.tensor_copy(
                                hrow4[:, j, 288:320],
                                hp[:, 257:258].to_broadcast([128, 32]))
                        nc.sync.dma_start(hpad_v[g], hrow4[:])

                    # edge chunks: gather, scores, one-hot matmul aggregate
                    for ch in range(NCHUNK):
                        icol = ch * (CE // 16)
                        g2 = gcp.tile([128, CHUNK_KT, 64], F32, tag="g2")
                        nc.gpsimd.dma_gather(
                            g2[:], hpad_d[:, 256:320],
                            gdst[:, icol:icol + CE // 16],
                            CE, CE, 64, elem_step=TW)
                        g1 = gcp.tile([128, CHUNK_KT, TW], F32, tag="g1")
                        nc.gpsimd.dma_gather(
                            g1[:], hpad_d[:],
                            gsrc[:, icol:icol + CE // 16],
                            CE, CE, TW)
                        # score = a_s[src] + a_d[dst]; ex = exp(lrelu(score))
                        sc = wp.tile([128, CHUNK_KT], F32, tag="sc")
                        nc.vector.tensor_tensor(
                            sc[:], g1[:, :, 256], g2[:, :, 32], op=OP.add)
                        lr = wp.tile([128, CHUNK_KT], F32, tag="lr")
                        nc.vector.tensor_scalar(
                            lr[:], sc[:], NEG, None, op0=OP.mult)
                        nc.vector.tensor_tensor(
                            lr[:], lr[:], sc[:], op=OP.max)
                        nc.scalar.activation(sc[:], lr[:], AF.Exp)
                        # rhs col 256 := 1 so psum col 256 sums the lhsT's
                        # ex weights -> segment softmax denominator
                        nc.vector.tensor_copy(g1[:, :, 256], ones16[:])

                        for t16 in range(CHUNK_KT):
                            t = ch * CHUNK_KT + t16
                            b = t // KPB
                            if t % KPB == 0:
                                agg = psp.tile([128, 257], F32, tag="agg")
                            at = wp.tile([128, 128], F32, tag="at")
                            nc.vector.tensor_scalar(
                                at[:], d1h[:, t, :], sc[:, t16:t16 + 1], None,
                                op0=OP.mult)
                            nc.tensor.matmul(
                                agg[:], at[:], g1[:, t16, :257],
                                start=(t % KPB == 0), stop=(t % KPB == KPB - 1))
                            if t % KPB == KPB - 1:
                                rcp = wp.tile([128, 1], F32, tag="rcp")
                                nc.vector.reciprocal(rcp[:], agg[:, 256:257])
                                xn = wp.tile([128, 256], F32, tag="xn")
                                nc.vector.tensor_scalar(
                                    xn[:], agg[:, :256], rcp[:], None,
                                    op0=OP.mult)
                                nc.vector.tensor_tensor(
                                    xn[:], xn[:], brep[:, l, :], op=OP.add)
                                for fb in range(2):
                                    tp = psp.tile([128, 128], F32, tag="tp")
                                    nc.tensor.transpose(
                                        tp[:], xn[:, fb * 128:(fb + 1) * 128],
                                        ident[:])
                                    nc.vector.tensor_copy(
                                        xTown[:, fb, b * 128:(b + 1) * 128],
                                        tp[:])
                    if l < L - 1:
                        nc.sync.dma_start(ccin_d[:], xTown[:])
                        nc.gpsimd.collective_compute(
                            "AllGather", mybir.AluOpType.bypass,
                            replica_groups=[list(range(NCORES))],
                            ins=[ccin_d[:]], outs=[ccout_d[:]])
                        nc.sync.dma_start(
                            xg[:],
                            ccout_d[:].rearrange("(c p) k n -> p c k n", p=128))

            # ---------------- FC + gumbel + scatter phase ----------------
            with tc.tile_pool(name="fc", bufs=1) as fcp, \
                 tc.tile_pool(name="fw", bufs=2) as fwp, \
                 tc.tile_pool(name="fpsum", bufs=2, space="PSUM") as fps:
                dw = fcp.tile([128, 2, NPP], F32)
                nc.sync.dma_start(dw[:], dW_d[:])
                db = fcp.tile([128, NPP // 128], F32)
                nc.sync.dma_start(db[:], db_d[:])
                s0 = fcp.tile([128, NPP // 16], I16)
                nc.sync.dma_start(s0[:], s0_d[:])
                s1 = fcp.tile([128, NPP // 16], I16)
                nc.sync.dma_start(s1[:], s1_d[:])
                gT_v = gT_d[:].rearrange("(pb p) k n -> pb p k n", p=128)

                for sc4 in range(4):            # scatter groups of 16 pb
                    adjc = fwp.tile([128, 16, NPC], F32, tag="adjc")
                    for i16 in range(16):
                        pb = sc4 * 16 + i16
                        fcps = fps.tile([128, NPC], F32, tag="fcps")
                        for k in range(2):
                            nc.tensor.matmul(
                                fcps[:], dw[:, k, pb * 128:(pb + 1) * 128],
                                xTown[:, k, :],
                                start=(k == 0), stop=(k == 1))
                        gch = fwp.tile([128, 2, NPC], F32, tag="gch")
                        nc.sync.dma_start(gch[:], gT_v[pb])
                        gf = gch[:].rearrange("p k n -> p (k n)")
                        ln1 = fwp.tile([128, 2 * NPC], F32, tag="ln1")
                        nc.scalar.activation(ln1[:], gf, AF.Ln,
                                             bias=epsb[:], scale=1.0)
                        ln2 = fwp.tile([128, 2, NPC], F32, tag="ln2")
                        ln2f = ln2[:].rearrange("p k n -> p (k n)")
                        nc.scalar.activation(ln2f, ln1[:], AF.Ln,
                                             bias=epsb[:], scale=-1.0)
                        dl = fwp.tile([128, NPC], F32, tag="dl")
                        nc.vector.tensor_tensor(
                            dl[:], ln2[:, 1, :], ln2[:, 0, :], op=OP.subtract)
                        nc.vector.tensor_tensor(
                            dl[:], dl[:], fcps[:], op=OP.add)
                        nc.vector.tensor_scalar(
                            dl[:], dl[:], db[:, pb:pb + 1], None, op0=OP.add)
                        nc.vector.tensor_scalar(
                            adjc[:, i16, :], dl[:], 0.0, None, op0=OP.is_ge)
                    icol = sc4 * (2048 // 16)
                    nvalid = 2048 if sc4 < 3 else NP - 3 * 2048
                    nc.gpsimd.dma_scatter_add(
                        A_d[:], adjc[:], s0[:, icol:icol + 128],
                        2048, nvalid, NPC)
                    nc.gpsimd.dma_scatter_add(
                        A_d[:], adjc[:], s1[:, icol:icol + 128],
                        2048, nvalid, NPC)

    nc.compile()
    return nc


def _get_program():
    if "nc" not in _prog_cache:
        _prog_cache["nc"] = _build_program()
    return _prog_cache["nc"]


def _host_prep(x, edge_index, gumbel_u, gat_W, gat_a_src, gat_a_dst, gat_b,
               fc_W, fc_b):
    f32 = np.float32
    x = np.asarray(x, f32)
    ei = np.asarray(edge_index, np.int64)
    gat_W = np.asarray(gat_W, f32)
    gat_a_src = np.asarray(gat_a_src, f32)
    gat_a_dst = np.asarray(gat_a_dst, f32)
    gat_b = np.asarray(gat_b, f32)
    fc_W = np.asarray(fc_W, f32)
    fc_b = np.asarray(fc_b, f32)

    # shared (replicated) inputs
    xT0 = np.ascontiguousarray(
        x.reshape(NCORES, BLK, 128, 2, 128).transpose(4, 0, 3, 1, 2)
        .reshape(128, NCORES, 2, NPC))
    Waug = np.empty((128, L, 2, 258), f32)
    brep = np.empty((128, L, 256), f32)
    for l in range(L):
        wa = np.concatenate(
            [gat_W[l], (gat_W[l] @ gat_a_src[l])[:, None],
             (gat_W[l] @ gat_a_dst[l])[:, None]], axis=1)   # [256, 258]
        Waug[:, l] = wa.reshape(2, 128, 258).transpose(1, 0, 2)
        brep[:, l] = gat_b[l][None, :]
    dwm = fc_W[:, 0::2] - fc_W[:, 1::2]                     # [256, 8128]
    dW = np.zeros((128, 2, NPP), f32)
    dW[:, :, :NP] = dwm.reshape(2, 128, NP).transpose(1, 0, 2)
    dbv = np.zeros(NPP, f32)
    dbv[:NP] = fc_b[0::2] - fc_b[1::2]
    db = dbv.reshape(NPP // 128, 128).T.copy()              # [128, 64]

    iu, ju = np.triu_indices(NN, 1)
    sidx0 = np.full(NPP, -1, np.int64)
    sidx0[:NP] = iu * NN + ju
    sidx1 = np.full(NPP, -1, np.int64)
    sidx1[:NP] = ju * NN + iu
    s0 = _wrap16(sidx0, NPP // 16)
    s1 = _wrap16(sidx1, NPP // 16)

    # per-core edge structures
    loops = np.arange(B, dtype=np.int64)
    src_g = np.concatenate([ei[0], loops])
    dst_g = np.concatenate([ei[1], loops])
    in_maps = []
    for c in range(NCORES):
        m = (dst_g >= c * NPC) & (dst_g < (c + 1) * NPC)
        s_c, d_c = src_g[m], dst_g[m]
        order = np.argsort(d_c, kind="stable")
        s_c, d_c = s_c[order], d_c[order]
        slots_src = np.zeros(NPAD, np.int64)
        slots_dst = np.zeros(NPAD, np.int64)
        dloc = np.zeros(NPAD, np.int64)
        valid = np.zeros(NPAD, f32)
        for b in range(BLK):
            lo, hi = c * NPC + b * 128, c * NPC + (b + 1) * 128
            bm = (d_c >= lo) & (d_c < hi)
            k = int(bm.sum())
            assert k <= KPB * 128, f"dst block overflow: {k}"
            base = b * KPB * 128
            slots_src[base:base + k] = s_c[bm]
            slots_dst[base:base + k] = d_c[bm]
            dloc[base:base + k] = d_c[bm] - lo
            valid[base:base + k] = 1.0
        e = np.arange(NPAD)
        D1h = np.zeros((128, KT, 128), f32)
        D1h[e % 128, e // 128, dloc] = valid
        gsrc = _wrap16(slots_src, NPAD // 16)
        gdst = _wrap16(slots_dst, NPAD // 16)
        gu_c = np.asarray(gumbel_u[c * NPC:(c + 1) * NPC], f32)
        gT = np.zeros((NPP, 2, NPC), f32)
        gT[:NP] = gu_c.transpose(1, 2, 0)
        gT[NP:] = 0.5
        in_maps.append({
            "xT0": xT0, "Waug": Waug, "brep": brep, "dW": dW, "db": db,
            "D1h": D1h, "gsrc": gsrc, "gdst": gdst, "s0": s0, "s1": s1,
            "gT": np.ascontiguousarray(gT),
        })
    return in_maps


def run(inputs, trace=False, trace_cores=None):
    from concourse.bass_utils import run_bass_kernel_spmd
    nc = _get_program()
    in_maps = _host_prep(**inputs)
    res = run_bass_kernel_spmd(
        nc, in_maps, list(range(NCORES)),
        trace=trace, trace_cores=trace_cores)
    A = np.empty((B, NN, NN), np.float32)
    for c in range(NCORES):
        ap = res.results[c]["Aperm"]          # [NN*NN, NPC]
        A[c * NPC:(c + 1) * NPC] = ap.T.reshape(NPC, NN, NN)
    return A, res


def kernel(**inputs):
    A, _ = run(inputs, trace=False)
    return A
